# revision 1
# baseline (speedup 1.0000x reference)
"""Distributed KNN-cache retrieval kernel for 8 Trainium2 NeuronCores.

Reference computation (B=32 queries, N=20000 cache entries, k=8):
    scores = query @ keys.T / sqrt(512)        [B, N]
    attn   = softmax(scores, axis=-1)
    topk_w, topk_idx = top_k(attn, 8)          [B*8]
    outputs = values[topk_idx]                 [B*8, 20, 512]

Distribution (classic distributed ANN/KNN):
  Phase A (SPMD, 8 cores): keys sharded along N (2500 rows/core). Each core
    computes its score shard with TensorE (fp32), per-512-chunk top-8
    values+indices with the DVE Max8/MaxIndex instructions, and per-chunk
    sum(exp(score)) with ScalarE (fused accumulate) for the softmax
    denominator.
  Host: merges the 8*5*8 candidates per query to the global top-8, combines
    the partial softmax denominators, computes the 256 softmax weights.
  Phase B (SPMD, 8 cores): values replicated; each core gathers 32 of the
    256 selected [20, 512] value rows via indirect DMA and writes its slice
    of the output.
"""

import math

import numpy as np

import concourse.bacc as bacc
import concourse.mybir as mybir
import concourse.tile as tile
from concourse import bass
from concourse.bass_utils import run_bass_kernel_spmd

F32 = mybir.dt.float32
U32 = mybir.dt.uint32
I32 = mybir.dt.int32

N_CORES = 8
B = 32          # queries
N = 20000       # cache entries
DK = 512        # key dim
L, DV = 20, 512  # value row shape
TOPK = 8
NSH = N // N_CORES          # 2500 keys per core
NB = 5                      # score chunks per core
CH = NSH // NB              # 500 columns per chunk (fits one PSUM bank)
DC = DK // 128              # 4 contraction tiles

# Phase B layout: one value row [20, 512] = 10240 f32 split across 4
# partitions of 2560 f32 so a [128, 2560] gather covers 32 rows at full
# DMA-port parallelism.
ROW_SPLIT = 4
VCOLS = L * DV // ROW_SPLIT  # 2560


def build_phase_a():
    nc = bacc.Bacc("TRN2", target_bir_lowering=False, debug=False,
                   num_devices=N_CORES)
    qt = nc.dram_tensor("qt", [128, DC, B], F32, kind="ExternalInput")
    kt = nc.dram_tensor("kt", [NB, 128, DC, CH], F32, kind="ExternalInput")
    o_cv = nc.dram_tensor("cv", [B, NB, TOPK], F32, kind="ExternalOutput")
    o_ci = nc.dram_tensor("ci", [B, NB, TOPK], U32, kind="ExternalOutput")
    o_se = nc.dram_tensor("se", [B, NB], F32, kind="ExternalOutput")

    with tile.TileContext(nc) as tc:
        with (
            tc.tile_pool(name="persist", bufs=1) as ppool,
            tc.tile_pool(name="keys", bufs=3) as kpool,
            tc.tile_pool(name="sc", bufs=2) as spool,
            tc.tile_pool(name="ex", bufs=2) as epool,
            tc.tile_pool(name="ps", bufs=4, space="PSUM") as pspool,
        ):
            q_sb = ppool.tile([128, DC, B], F32)
            nc.sync.dma_start(out=q_sb[:], in_=qt.ap())
            cv_sb = ppool.tile([B, NB, TOPK], F32)
            ci_sb = ppool.tile([B, NB, TOPK], U32)
            se_sb = ppool.tile([B, NB], F32)

            for nb in range(NB):
                k_sb = kpool.tile([128, DC, CH], F32, tag="keys")
                nc.sync.dma_start(out=k_sb[:], in_=kt.ap()[nb])
                ps = pspool.tile([B, CH], F32, tag="ps")
                for dc in range(DC):
                    nc.tensor.matmul(
                        ps[:],
                        lhsT=q_sb[:, dc, :],
                        rhs=k_sb[:, dc, :],
                        start=(dc == 0),
                        stop=(dc == DC - 1),
                    )
                sc = spool.tile([B, CH], F32, tag="sc")
                nc.scalar.copy(sc[:], ps[:])
                nc.vector.max(out=cv_sb[:, nb, :], in_=sc[:])
                nc.vector.max_index(
                    out=ci_sb[:, nb, :], in_max=cv_sb[:, nb, :], in_values=sc[:]
                )
                ex = epool.tile([B, CH], F32, tag="ex")
                nc.scalar.activation(
                    ex[:], sc[:], mybir.ActivationFunctionType.Exp,
                    accum_out=se_sb[:, nb:nb + 1],
                )

            nc.sync.dma_start(out=o_cv.ap(), in_=cv_sb[:])
            nc.sync.dma_start(out=o_ci.ap(), in_=ci_sb[:])
            nc.sync.dma_start(out=o_se.ap(), in_=se_sb[:])
    nc.compile()
    return nc


def build_phase_b():
    nc = bacc.Bacc("TRN2", target_bir_lowering=False, debug=False,
                   num_devices=N_CORES)
    vals = nc.dram_tensor("vals", [N * ROW_SPLIT, VCOLS], F32,
                          kind="ExternalInput")
    idx = nc.dram_tensor("idx", [128, 1], I32, kind="ExternalInput")
    out = nc.dram_tensor("out", [128, VCOLS], F32, kind="ExternalOutput")

    with tile.TileContext(nc) as tc:
        with tc.tile_pool(name="g", bufs=1) as pool:
            idx_sb = pool.tile([128, 1], I32)
            nc.sync.dma_start(out=idx_sb[:], in_=idx.ap())
            g = pool.tile([128, VCOLS], F32)
            nc.gpsimd.indirect_dma_start(
                out=g[:],
                out_offset=None,
                in_=vals.ap(),
                in_offset=bass.IndirectOffsetOnAxis(ap=idx_sb[:, :1], axis=0),
            )
            nc.sync.dma_start(out=out.ap(), in_=g[:])
    nc.compile()
    return nc


_NC_CACHE = {}


def _get_nc(name):
    if name not in _NC_CACHE:
        _NC_CACHE[name] = {"a": build_phase_a, "b": build_phase_b}[name]()
    return _NC_CACHE[name]


def host_prepare(query, keys):
    """Host-side input staging: fold the 1/sqrt(dk) scale into the query,
    transpose + tile both operands into the DMA-friendly layouts."""
    scale = 1.0 / math.sqrt(DK)
    # qt[p, dc, b] = query[b, dc*128+p] * scale
    qt = np.ascontiguousarray(
        (query.astype(np.float32) * scale).reshape(B, DC, 128).transpose(2, 1, 0)
    )
    # kt_c[nb, p, dc, n] = keys[c*NSH + nb*CH + n, dc*128+p]
    kts = []
    for c in range(N_CORES):
        shard = keys[c * NSH:(c + 1) * NSH]  # [NSH, DK]
        kt = np.ascontiguousarray(
            shard.reshape(NB, CH, DC, 128).transpose(0, 3, 2, 1)
        )
        kts.append(kt)
    return qt, kts


def host_merge(cvs, cis, ses):
    """Merge per-core per-chunk candidates into the global top-8 per query.

    cvs/cis: [N_CORES][B, NB, TOPK] candidate scores / chunk-local indices
    ses:     [N_CORES][B, NB] per-chunk sum(exp(score))
    Returns (topk_w [B*TOPK] f32, flat_idx [B*TOPK] int64).
    """
    vals = np.stack(cvs).astype(np.float64)     # [C, B, NB, K]
    idxs = np.stack(cis).astype(np.int64)       # [C, B, NB, K]
    # globalize indices: core c, chunk nb, local n -> c*NSH + nb*CH + n
    base = (np.arange(N_CORES)[:, None, None, None] * NSH
            + np.arange(NB)[None, None, :, None] * CH)
    gidx = (idxs + base).transpose(1, 0, 2, 3).reshape(B, -1)   # [B, C*NB*K]
    gval = vals.transpose(1, 0, 2, 3).reshape(B, -1)

    denom = np.stack(ses).astype(np.float64).sum(axis=(0, 2))   # [B]

    topk_w = np.empty((B, TOPK), np.float64)
    top_idx = np.empty((B, TOPK), np.int64)
    for b in range(B):
        order = np.lexsort((gidx[b], -gval[b]))[:TOPK]
        top_idx[b] = gidx[b][order]
        topk_w[b] = np.exp(gval[b][order]) / denom[b]
    return topk_w.reshape(-1).astype(np.float32), top_idx.reshape(-1)


def kernel(query, keys, values):
    query = np.asarray(query, dtype=np.float32)
    keys = np.asarray(keys, dtype=np.float32)
    values = np.asarray(values, dtype=np.float32)

    qt, kts = host_prepare(query, keys)
    nc_a = _get_nc("a")
    in_maps = [{"qt": qt, "kt": kts[c]} for c in range(N_CORES)]
    res_a = run_bass_kernel_spmd(nc_a, in_maps, core_ids=list(range(N_CORES)))
    cvs = [res_a.results[c]["cv"] for c in range(N_CORES)]
    cis = [res_a.results[c]["ci"] for c in range(N_CORES)]
    ses = [res_a.results[c]["se"] for c in range(N_CORES)]

    topk_w, flat_idx = host_merge(cvs, cis, ses)

    # Phase B: core c gathers output slots [c*32, (c+1)*32).
    nc_b = _get_nc("b")
    vals_view = values.reshape(N * ROW_SPLIT, VCOLS)
    in_maps_b = []
    for c in range(N_CORES):
        rows = flat_idx[c * 32:(c + 1) * 32]
        idx4 = (rows[:, None] * ROW_SPLIT
                + np.arange(ROW_SPLIT)[None, :]).reshape(128, 1)
        in_maps_b.append({"vals": vals_view,
                          "idx": idx4.astype(np.int32)})
    res_b = run_bass_kernel_spmd(nc_b, in_maps_b, core_ids=list(range(N_CORES)))
    outputs = np.concatenate(
        [res_b.results[c]["out"].reshape(32, L, DV) for c in range(N_CORES)]
    )
    return topk_w, outputs
